# revision 49
# baseline (speedup 1.0000x reference)
"""Trainium2 Bass kernel for nn_BoundaryDiceLoss_82171314307268.

Sharding: pure data-parallel over 8 cores; core c handles sample c//2,
D-half c%2. Each core gets a [H=128(partitions), 70 D-slots, w] slab
(64 owned D slices + 3 halo, out-of-volume D replicated with edge
values).

Host prep (per core):
  v   = (diff > 0) + (63*t + 1) in {1,2,64,65}, bf16, packed
        [128, 70, 132] with replicated w-pad cols 1/130 (cols 0/131
        dead). Carry-freedom of this value set: the 6-neighbor sum
        equals 6*center iff all 6 neighbors equal the center.
  dif = out1 - out0 (owned slots only), bf16 [128, 64*128]
  u   = UK*t or 1 in {1, 4096}, bf16 [128, 64*128]

Device pipeline, emitted in paired 8-slot blocks with a 2-deep skew so
Tile software-pipelines DMA / PE / ACT / DVE across the whole kernel:
  E     = (c_v - 6v)^2: 4 matmuls per 4-slot chunk (banded m_b = A1-6I
          with replicated H edges, I @ w-shifts, I @ tbv) where
          tbv = v[z-1]+v[z+1] is the one DVE add; two chunks share a
          2-bank PSUM tile and one FD-1024 ACT Square evacuation.
  r     = conv3d(E, ball radius 2), 8 matmuls per chunk:
          T5@E + T3@s3z + T3@f[w-1] + T3@f[w+1]
          + I@E[w-2] + I@E[w+2] + I@E[z-2] + I@E[z+2]
          with s3z = E[z-1]+E[z+1], f = E + s3z computed on DVE at
          16-slot granularity; matmuls grouped by stationary weight,
          FD-1024 ACT copy evacuates each pair to r3 (bf16).
  products, 3 accumulating STT passes per span (these run in DVE 1x
  mode -- accumulation caps the perf mode -- so spans are large:
  3 x FD-2048 + 2 x FD-1024 at the tail to keep the pipe short):
          z1 = (r>0.5)*probs   -> S1 = sum probs*m
          z2 = z1*u            -> B  = (UK-1)*S2 + S1
          z3 = (r>0.5)*u       -> A  = UK*S3 + (S4 - S3)
          (A is exact in f32: integer-valued <= 4096*2048 + 2048 <
           2^24; per-(partition,span) decode S3 = A//UK.)
  probs = sigmoid(dif) on ACT, FD-2048 blocks.
Host decodes the [128, 15] f32 partials and does the dice math.
"""
import sys

sys.path.insert(0, "/opt/trn_rl_repo")

import numpy as np
import ml_dtypes

import concourse.bass as bass
import concourse.bacc as bacc
import concourse.tile as tile
import concourse.mybir as mybir
from concourse.bass_utils import run_bass_kernel_spmd

f32 = mybir.dt.float32
bf16 = mybir.dt.bfloat16
fp16 = mybir.dt.float16
Alu = mybir.AluOpType
Act = mybir.ActivationFunctionType

P = 128          # H on partitions
W = 128
OWN = 64         # owned D slices per core
HALO = 3
DEXT = OWN + 2 * HALO          # 70 slab D-slots
WP = W + 4                     # padded w stride, data cols [2, 130)
B = 4
EPS = 1e-05

CH = 4                         # D-slots per chunk (512 free elems)
NE = 17                        # E chunks (slots 1..68)
ND = 16                        # dilation chunks (owned 3..66)
NG = 4                         # product groups (16 slots / FD 2048 each)
UK = 4096.0                    # u = UK*t + 1 count-packing constant


def _band(offsets, rep_edges=False):
    m = np.zeros((P, P), np.float32)
    for o in offsets:
        for i in range(P):
            j = i + o
            if 0 <= j < P:
                m[j, i] += 1.0
            elif rep_edges:
                m[min(max(j, 0), P - 1), i] += 1.0
    return m


def _const_mats():
    a1 = _band([-1, 1], rep_edges=True)   # H-neighbor sum, edges replicated
    m_b = a1 - 6.0 * np.eye(P, dtype=np.float32)
    return {"m_b": m_b, "m_t3": _band([-1, 0, 1]),
            "m_t5": _band([-2, -1, 0, 1, 2]),
            "m_id": np.eye(P, dtype=np.float32)}


def _build_program():
    nc = bacc.Bacc("TRN2", target_bir_lowering=False, debug=False,
                   num_devices=8)
    d_v = nc.dram_tensor("v", [P, DEXT * WP], bf16, kind="ExternalInput")
    d_dif = nc.dram_tensor("dif", [P, OWN * W], bf16, kind="ExternalInput")
    d_u = nc.dram_tensor("u", [P, OWN * W], bf16, kind="ExternalInput")
    d_mats = nc.dram_tensor("mats", [P, 4 * P], bf16, kind="ExternalInput")
    d_psums = nc.dram_tensor("psums", [P, 24], f32, kind="ExternalOutput")

    with tile.TileContext(nc) as tc:
        with tc.tile_pool(name="consts", bufs=1) as cp, \
             tc.tile_pool(name="slabs", bufs=1) as sp, \
             tc.tile_pool(name="scr", bufs=3) as zp, \
             tc.tile_pool(name="ps_e", bufs=2, space="PSUM") as ps_e, \
             tc.tile_pool(name="ps_p", bufs=2, space="PSUM") as ps_p:

            mt = cp.tile([P, 4 * P], bf16, tag="mats", name="mats")
            nc.scalar.dma_start(mt[:], d_mats[:])
            mats = {"m_b": mt[:, 0:128], "m_id": mt[:, 128:256],
                    "m_t3": mt[:, 256:384], "m_t5": mt[:, 384:512]}

            def slab(name_, cols=WP, dtype=bf16, slots=DEXT):
                t = sp.tile([P, slots * cols], dtype, tag=name_, name=name_)
                return t, t.rearrange("p (s w) -> p s w", w=cols)

            vflat, v3 = slab("v")
            _, e3 = slab("e")
            _, s3z = slab("s3z")
            _, f3 = slab("f")
            _, tbv = slab("tbv", cols=W, slots=68)
            pflat, probs = slab("probs", cols=W, slots=OWN)
            rflat, r3 = slab("r", cols=W, slots=OWN)
            uflat, u3 = slab("u", cols=W, dtype=bf16, slots=OWN)
            difflat, dif3 = slab("dif", cols=W, slots=OWN)
            acc = sp.tile([P, 24], f32, tag="acc")  # 8 pairs x [S1,B,A]

            # zero E w-pads once (dilation must see 0 out-of-volume;
            # s3z/f inherit zeros from e3's pads automatically)
            nc.vector.memset(e3[:, :, 0:2], 0.0)
            nc.vector.memset(e3[:, :, 130:132], 0.0)

            # ---- input DMA: v first (E-phase gates on it), dif/u after ----
            vbnd = [0, 6, 14, 22, 30, 38, 46, 54, 62, 70]

            def dma_v(k):
                s0, s1 = vbnd[k], vbnd[k + 1]
                nc.sync.dma_start(vflat[:, s0 * WP:s1 * WP],
                                  d_v[:, s0 * WP:s1 * WP])

            def dma_du(k):
                s0, s1 = 16 * k * W, 16 * (k + 1) * W
                nc.sync.dma_start(difflat[:, s0:s1], d_dif[:, s0:s1])
                nc.sync.dma_start(uflat[:, s0:s1], d_u[:, s0:s1])

            for k in (0, 1, 2):
                dma_v(k)
            dma_du(0)
            dma_v(3)
            dma_v(4)
            dma_du(1)
            dma_v(5)
            dma_v(6)
            dma_du(2)
            dma_v(7)
            dma_v(8)
            dma_du(3)

            def e_mms_pair(pe3s, gs):
                # same-weight matmuls grouped across the two PSUM halves
                for pe3, g in zip(pe3s, gs):
                    sl = slice(1 + CH * g, 5 + CH * g)
                    nc.tensor.matmul(pe3[:], mats["m_b"], v3[:, sl, 2:130],
                                     start=True, stop=False)
                for pe3, g in zip(pe3s, gs):
                    sl = slice(1 + CH * g, 5 + CH * g)
                    g0 = CH * g
                    nc.tensor.matmul(pe3[:], mats["m_id"], v3[:, sl, 1:129],
                                     start=False, stop=False)
                    nc.tensor.matmul(pe3[:], mats["m_id"], v3[:, sl, 3:131],
                                     start=False, stop=False)
                    nc.tensor.matmul(pe3[:], mats["m_id"],
                                     tbv[:, g0:g0 + CH, :],
                                     start=False, stop=True)

            def e_pair(h):
                # tbv = v[z-1] + v[z+1]; first pair split so chunk 0 only
                # needs the first v DMA group
                if h == 0:
                    nc.vector.tensor_add(tbv[:, 0:4, :], v3[:, 0:4, 2:130],
                                         v3[:, 2:6, 2:130])
                    nc.vector.tensor_add(tbv[:, 4:8, :], v3[:, 4:8, 2:130],
                                         v3[:, 6:10, 2:130])
                else:
                    g0 = 8 * h
                    n = min(8, 68 - g0)
                    nc.vector.tensor_add(tbv[:, g0:g0 + n, :],
                                         v3[:, g0:g0 + n, 2:130],
                                         v3[:, g0 + 2:g0 + n + 2, 2:130])
                pe_ = ps_e.tile([P, 2 * CH * W], f32, tag="eps")
                if h < 8:
                    pea = pe_[:, 0:CH * W].rearrange("p (s w) -> p s w", w=W)
                    peb = pe_[:, CH * W:].rearrange("p (s w) -> p s w", w=W)
                    e_mms_pair([pea, peb], [2 * h, 2 * h + 1])
                    sl2 = slice(1 + 8 * h, 9 + 8 * h)
                    nc.scalar.activation(e3[:, sl2, 2:130],
                                         pe_[:].rearrange(
                                             "p (s w) -> p s w", w=W),
                                         Act.Square)
                else:  # last solo chunk 16
                    pea = pe_[:, 0:CH * W].rearrange("p (s w) -> p s w", w=W)
                    e_mms_pair([pea], [16])
                    sl2 = slice(65, 69)
                    nc.scalar.activation(e3[:, sl2, 2:130], pea[:],
                                         Act.Square)

            def dil_pair(q):
                t0 = 3 + 8 * q
                tq = slice(t0, t0 + 8)
                nc.vector.tensor_add(s3z[:, tq, :],
                                     e3[:, t0 - 1:t0 + 7, :],
                                     e3[:, t0 + 1:t0 + 9, :])
                nc.vector.tensor_add(f3[:, tq, :], e3[:, tq, :],
                                     s3z[:, tq, :])
                pp = ps_p.tile([P, 2 * CH * W], f32, tag="pps")
                halves = [pp[:, 0:CH * W].rearrange("p (s w) -> p s w", w=W),
                          pp[:, CH * W:].rearrange("p (s w) -> p s w", w=W)]
                sls = [slice(3 + CH * j, 3 + CH * j + CH)
                       for j in (2 * q, 2 * q + 1)]
                for pp3, sl in zip(halves, sls):
                    nc.tensor.matmul(pp3[:], mats["m_t5"], e3[:, sl, 2:130],
                                     start=True, stop=False)
                for pp3, sl in zip(halves, sls):
                    nc.tensor.matmul(pp3[:], mats["m_t3"], s3z[:, sl, 2:130],
                                     start=False, stop=False)
                    nc.tensor.matmul(pp3[:], mats["m_t3"], f3[:, sl, 1:129],
                                     start=False, stop=False)
                    nc.tensor.matmul(pp3[:], mats["m_t3"], f3[:, sl, 3:131],
                                     start=False, stop=False)
                for pp3, sl in zip(halves, sls):
                    s0 = sl.start
                    nc.tensor.matmul(pp3[:], mats["m_id"], e3[:, sl, 0:128],
                                     start=False, stop=False)
                    nc.tensor.matmul(pp3[:], mats["m_id"], e3[:, sl, 4:132],
                                     start=False, stop=False)
                    nc.tensor.matmul(pp3[:], mats["m_id"],
                                     e3[:, s0 - 2:s0 + CH - 2, 2:130],
                                     start=False, stop=False)
                    nc.tensor.matmul(pp3[:], mats["m_id"],
                                     e3[:, s0 + 2:s0 + CH + 2, 2:130],
                                     start=False, stop=True)
                jj = slice(8 * q, 8 * q + 8)
                nc.scalar.copy(r3[:, jj, :],
                               pp[:].rearrange("p (s w) -> p s w", w=W))

            def prod_span(col, lo, hi):
                jj = slice(lo * W, hi * W)
                n = (hi - lo) * W
                z1 = zp.tile([P, n], bf16, tag="z1")
                z23 = zp.tile([P, n], f32, tag="z23")
                nc.vector.scalar_tensor_tensor(
                    z1[:], rflat[:, jj], 0.5, pflat[:, jj],
                    op0=Alu.is_gt, op1=Alu.mult,
                    accum_out=acc[:, col:col + 1])
                nc.vector.scalar_tensor_tensor(
                    z23[:], z1[:], 0.0, uflat[:, jj],
                    op0=Alu.add, op1=Alu.mult,
                    accum_out=acc[:, col + 1:col + 2])
                nc.vector.scalar_tensor_tensor(
                    z23[:], rflat[:, jj], 0.5, uflat[:, jj],
                    op0=Alu.is_gt, op1=Alu.mult,
                    accum_out=acc[:, col + 2:col + 3])

            # skewed emission: E pairs h=0..8, sigmoids s=0..3 (FD 2048),
            # dil pairs q=0..7, products (3 FD-2048 groups + 4 FD-512
            # singles to keep the tail short)
            for i in range(11):
                if i < 9:
                    e_pair(i)
                if i in (0, 2, 4, 6):
                    s = i // 2
                    cc = slice(16 * s * W, 16 * (s + 1) * W)
                    nc.scalar.activation(pflat[:, cc], difflat[:, cc],
                                         Act.Sigmoid)
                if 1 <= i < 9:
                    dil_pair(i - 1)
                if 3 <= i < 11:
                    p = i - 3      # FD-1024 product units, 2-iter slack
                    prod_span(3 * p, 8 * p, 8 * (p + 1))

            nc.sync.dma_start(d_psums[:], acc[:])

    nc.compile()
    return nc


_CACHE = {}
TRACE = False
TRACE_TMPDIR = None
_LAST = {"exec_time_ns": None, "results": None}


def _get_program():
    if "nc" not in _CACHE:
        _CACHE["nc"] = _build_program()
    return _CACHE["nc"]


def last_exec_time_ns():
    return _LAST["exec_time_ns"]


def _core_slabs(diff_p, tgt_p, c):
    s, h = c // 2, c % 2
    d0 = 0 if h == 0 else OWN
    sl = slice(d0, d0 + DEXT)

    def tr(a):  # [S,H,W] -> [H, S, W]
        return np.ascontiguousarray(a.transpose(1, 0, 2))

    tgt = tr(tgt_p[s][sl])                            # [H, 70, W] f32
    dfull = tr(diff_p[s][sl])                         # [H, 70, W]
    dif = dfull[:, HALO:HALO + OWN]                   # [H, 64, W]
    v = (dfull > 0.0).astype(np.float32) + (63.0 * tgt + 1.0)
    vp = np.zeros((P, DEXT, WP), np.float32)
    vp[:, :, 2:130] = v
    vp[:, :, 1] = v[:, :, 0]
    vp[:, :, 130] = v[:, :, -1]
    u = np.where(tgt[:, HALO:HALO + OWN] > 0.5, UK, 1.0)  # {1, UK}
    return {
        "v": vp.reshape(P, DEXT * WP).astype(ml_dtypes.bfloat16),
        "dif": dif.reshape(P, OWN * W).astype(ml_dtypes.bfloat16),
        "u": u.reshape(P, OWN * W).astype(ml_dtypes.bfloat16),
    }


def kernel(output, target):
    output = np.asarray(output, dtype=np.float32)
    target = np.asarray(target, dtype=np.float32)
    nc = _get_program()

    diff = output[:, 1] - output[:, 0]                # [B, D, H, W]
    diff_p = np.pad(diff, ((0, 0), (HALO, HALO), (0, 0), (0, 0)),
                    mode="edge")
    tgt_p = np.pad(target[:, 0], ((0, 0), (HALO, HALO), (0, 0), (0, 0)),
                   mode="edge")

    cm = _const_mats()
    mats = np.concatenate(
        [cm["m_b"], cm["m_id"], cm["m_t3"], cm["m_t5"]], axis=1
    ).astype(ml_dtypes.bfloat16)
    in_maps = []
    for c in range(8):
        m = _core_slabs(diff_p, tgt_p, c)
        m["mats"] = mats
        in_maps.append(m)

    res = run_bass_kernel_spmd(nc, in_maps, list(range(8)), trace=TRACE,
                               tmpdir=TRACE_TMPDIR)
    _LAST["exec_time_ns"] = res.exec_time_ns
    _LAST["results"] = res

    s1 = np.zeros(B, np.float64)
    s2 = np.zeros(B, np.float64)
    s3 = np.zeros(B, np.float64)
    s4 = np.zeros(B, np.float64)
    for c in range(8):
        # u in {1, UK}: A = UK*S3 + (S4 - S3); B = (UK-1)*S2 + S1
        a = res.results[c]["psums"].astype(np.float64)   # [128, 24]
        S1 = a[:, 0::3].sum()
        Bv = a[:, 1::3].sum()
        A = a[:, 2::3]
        S3 = np.floor(A / UK).sum()
        S4 = (A - np.floor(A / UK) * UK).sum() + S3
        S2 = (Bv - S1) / (UK - 1.0)
        s1[c // 2] += S1
        s2[c // 2] += S2
        s3[c // 2] += S3
        s4[c // 2] += S4
    dice = (2.0 * s2 + EPS) / (s1 + s3 + EPS)
    per_sample = np.where(s4 > 0, 1.0 - dice, 0.0)
    return np.float32(per_sample.sum() / B)
